# revision 19
# baseline (speedup 1.0000x reference)
"""Single-head causal self-attention on 8 Trainium2 NeuronCores.

Reference computation (per batch b):
    k = x @ Wk.T ; q = x @ Wq.T ; v = x @ Wv.T
    wei = softmax(mask(q @ k.T / sqrt(H)))
    out = wei @ v

Strategy (v5):
  - Data parallel: B=256 across 8 cores (32 batches each), replicated
    weights, no cross-core comm.
  - Host-side layout prep: x pre-transposed to xT[b] = x[b].T ([C, T]) in
    bf16; weight-only products G = (Wq.T @ Wk) / sqrt(H) and WvT = Wv.T
    precomputed on host (weight repacking), bf16.  No on-device
    transposes.
  - Scores via the G-trick in [key s, query t] layout; bf16 matmuls with
    fp32 PSUM (total rel err ~4e-3, gate 2e-2).
  - Causal block structure: the fully-masked (s-hi, t-lo) block is never
    computed; the two diagonal 128x128 blocks get a triangular 0/1 mask.
  - Two batches per iteration; the four score blocks share one 2-bank
    PSUM tile so a single Exp covers the pair.
  - Softmax denominator via ones-columns appended to V; normalization =
    reciprocal (DVE) + Copy-with-per-partition-scale (ACT) from PSUM.
  - Output stored bf16, one 3D-AP DMA per batch, upcast on host.
  - The out-stage of pair p-1 is issued between z2 and v/scores of pair
    p, hiding the exp->mask latency and the z2 PSUM->SBUF copy latency.
"""

import numpy as np
import ml_dtypes

import concourse.bass as bass
import concourse.mybir as mybir
from concourse import bacc
import concourse.tile as tile
from concourse.bass_utils import run_bass_kernel_spmd

B, T, C, H = 256, 256, 384, 384
NCORES = 8
NB = B // NCORES
P = 128
CC = C // P  # 3 chunks of the embedding dim
SCALE = float(H) ** -0.5
F32 = mybir.dt.float32
BF16 = mybir.dt.bfloat16
VW = H + 8  # v width incl. ones columns (8 cols = 16B in bf16)

BF16_NP = ml_dtypes.bfloat16


def build_bass(nb: int = NB):
    nc = bacc.Bacc(
        "TRN2",
        target_bir_lowering=False,
        debug=False,
        enable_asserts=False,
        num_devices=NCORES,
    )
    xT_d = nc.dram_tensor("xT", [nb, C, T], BF16, kind="ExternalInput").ap()
    g_d = nc.dram_tensor("G", [C, C], BF16, kind="ExternalInput").ap()
    wvT_d = nc.dram_tensor("WvT", [C, H], BF16, kind="ExternalInput").ap()
    out_d = nc.dram_tensor("out", [nb, T, H], BF16, kind="ExternalOutput").ap()

    npairs = nb // 2

    with tile.TileContext(nc) as tc:
        with (
            tc.tile_pool(name="const", bufs=1) as cpool,
            tc.tile_pool(name="sb", bufs=4) as sb,
            tc.tile_pool(name="ob", bufs=4) as obp,
            tc.tile_pool(name="pA", bufs=4, space="PSUM") as pA,
            tc.tile_pool(name="pB", bufs=2, space="PSUM") as pB,
        ):
            # triangular mask: trimask[p, t] = 1.0 where p <= t else 0.0
            trimask = cpool.tile([P, P], BF16, name="trimask")
            nc.gpsimd.memset(trimask, 1.0)
            nc.gpsimd.affine_select(
                out=trimask,
                in_=trimask,
                compare_op=mybir.AluOpType.is_ge,
                fill=0.0,
                base=0,
                channel_multiplier=-1,
                pattern=[[1, P]],
            )

            # G chunks [c1-part, C] and WvT chunks [c-part, H]
            g_s, wvT_s = [], []
            for cc_ in range(CC):
                gt = cpool.tile([P, C], BF16, name=f"g{cc_}")
                nc.scalar.dma_start(gt, g_d[cc_ * P : (cc_ + 1) * P, :])
                g_s.append(gt)
                wt = cpool.tile([P, H], BF16, name=f"wvT{cc_}")
                nc.scalar.dma_start(wt, wvT_d[cc_ * P : (cc_ + 1) * P, :])
                wvT_s.append(wt)

            def stage_z(p):
                """DMA + z2 for pair p."""
                b0 = 2 * p
                xT2 = []
                for cc_ in range(CC):
                    xt = sb.tile([P, 2 * T], BF16, name=f"x{cc_}", tag=f"x{cc_}")
                    nc.sync.dma_start(
                        xt.rearrange("p (two t) -> p two t", two=2),
                        xT_d[b0 : b0 + 2, cc_ * P : (cc_ + 1) * P, :].transpose(
                            [1, 0, 2]
                        ),
                    )
                    xT2.append(xt)

                # z2[c2] = G.T @ [xT_b0 | xT_b1]   ([128, 512], pre-scaled)
                z2 = []
                for c2 in range(CC):
                    pz = pA.tile([P, 512], F32, name="pz", tag="pA")[:, : 2 * T]
                    for c1 in range(CC):
                        nc.tensor.matmul(
                            pz,
                            lhsT=g_s[c1][:, c2 * P : (c2 + 1) * P],
                            rhs=xT2[c1],
                            start=(c1 == 0),
                            stop=(c1 == CC - 1),
                        )
                    zt = sb.tile([P, 2 * T], BF16, name=f"z{c2}", tag=f"z{c2}")
                    if c2 == 1:
                        nc.vector.tensor_copy(zt, pz)
                    else:
                        nc.scalar.activation(
                            zt, pz, mybir.ActivationFunctionType.Copy
                        )
                    z2.append(zt)
                return b0, xT2, z2

            def stage_v(st):
                """V projections for pair p (both s-blocks in one 2-bank tile)."""
                b0, xT2, z2 = st
                vau = []
                for i in range(2):
                    off = i * T
                    pv = pB.tile([P, 1024], F32, name="pv", tag="pB")
                    for sc in range(2):
                        dst = pv[:, sc * 512 : sc * 512 + H]
                        for cc_ in range(CC):
                            nc.tensor.matmul(
                                dst,
                                lhsT=xT2[cc_][:, off + sc * P : off + (sc + 1) * P],
                                rhs=wvT_s[cc_],
                                start=(cc_ == 0),
                                stop=(cc_ == CC - 1),
                            )
                    vt = sb.tile([P, 2 * VW], BF16, name=f"v{i}", tag=f"v{i}")
                    v3 = vt.rearrange("p (two w) -> p two w", two=2)
                    nc.vector.tensor_copy(
                        v3[:, :, :H],
                        pv.rearrange("p (two w) -> p two w", two=2)[:, :, :H],
                    )
                    nc.gpsimd.memset(v3[:, :, H:VW], 1.0)
                    vau.append(vt)
                return vau

            def stage_b(st):
                """Scores + exp + masks for pair p.

                Score blocks: pst[0:256) b0 diag+upper, [256:512) b1,
                [512:640) b0 lower diag, [640:768) b1 lower diag.
                """
                b0, xT2, z2 = st
                pst = pB.tile([P, 1024], F32, name="pst", tag="pB")
                for i in range(2):
                    off = i * T
                    d0 = pst[:, off : off + T]
                    for cc_ in range(CC):
                        nc.tensor.matmul(
                            d0,
                            lhsT=xT2[cc_][:, off : off + P],
                            rhs=z2[cc_][:, off : off + T],
                            start=(cc_ == 0),
                            stop=(cc_ == CC - 1),
                        )
                    d1 = pst[:, 512 + i * P : 512 + (i + 1) * P]
                    for cc_ in range(CC):
                        nc.tensor.matmul(
                            d1,
                            lhsT=xT2[cc_][:, off + P : off + T],
                            rhs=z2[cc_][:, off + P : off + T],
                            start=(cc_ == 0),
                            stop=(cc_ == CC - 1),
                        )
                est = sb.tile([P, 768], BF16, name="est", tag="est")
                nc.scalar.activation(
                    est, pst[:, :768], mybir.ActivationFunctionType.Exp
                )
                for i in range(2):
                    nc.vector.tensor_mul(
                        est[:, i * T : i * T + P], est[:, i * T : i * T + P], trimask
                    )
                    nc.vector.tensor_mul(
                        est[:, 512 + i * P : 512 + (i + 1) * P],
                        est[:, 512 + i * P : 512 + (i + 1) * P],
                        trimask,
                    )
                return est

            def out_stage(st):
                b0, vau, est = st
                for i in range(2):
                    off = i * T
                    va = vau[i]
                    po0 = pA.tile([P, 512], F32, name="po0", tag="pA")[:, :VW]
                    nc.tensor.matmul(
                        po0,
                        lhsT=est[:, off : off + P],
                        rhs=va[:, :VW],
                        start=True,
                        stop=True,
                    )
                    po1 = pA.tile([P, 512], F32, name="po1", tag="pA")[:, :VW]
                    nc.tensor.matmul(
                        po1,
                        lhsT=est[:, off + P : off + T],
                        rhs=va[:, :VW],
                        start=True,
                        stop=False,
                    )
                    nc.tensor.matmul(
                        po1,
                        lhsT=est[:, 512 + i * P : 512 + (i + 1) * P],
                        rhs=va[:, VW : 2 * VW],
                        start=False,
                        stop=True,
                    )
                    ot = obp.tile([P, 2 * H], BF16, name="ot", tag=f"ot{i}")
                    for tcc, po in ((0, po0), (1, po1)):
                        rec = obp.tile([P, 1], F32, name="rec", tag=f"rec{i}{tcc}")
                        nc.vector.reciprocal(rec, po[:, H : H + 1])
                        dst = ot[:, tcc * H : (tcc + 1) * H]
                        if tcc == 0:
                            nc.vector.tensor_scalar_mul(dst, po[:, :H], rec)
                        else:
                            nc.scalar.activation(
                                dst,
                                po[:, :H],
                                mybir.ActivationFunctionType.Copy,
                                scale=rec,
                            )
                    eng = nc.sync if i == 0 else nc.scalar
                    eng.dma_start(
                        out_d[b0 + i].rearrange("(two t) h -> t two h", two=2),
                        ot.rearrange("t (two h) -> t two h", two=2),
                    )

            prev = None
            for p in range(npairs):
                a = stage_z(p)
                if p == npairs - 1:
                    # final pair: scores first so the exp->mask latency
                    # overlaps the out-stage of p-1 and the v matmuls
                    est = stage_b(a)
                    out_stage(prev)
                    vau = stage_v(a)
                else:
                    if prev is not None:
                        out_stage(prev)
                    vau = stage_v(a)
                    est = stage_b(a)
                prev = (a[0], vau, est)
            out_stage(prev)

    nc.compile()
    return nc


_NC_CACHE = {}


def _get_nc(nb: int):
    if nb not in _NC_CACHE:
        _NC_CACHE[nb] = build_bass(nb)
    return _NC_CACHE[nb]


def kernel(x: np.ndarray, Wk: np.ndarray, Wq: np.ndarray, Wv: np.ndarray, **_):
    x = np.asarray(x, dtype=np.float32)
    Wk = np.asarray(Wk, dtype=np.float32)
    Wq = np.asarray(Wq, dtype=np.float32)
    Wv = np.asarray(Wv, dtype=np.float32)
    # host-side layout prep: transpose x per batch, weight-only products
    xT = np.ascontiguousarray(x.transpose(0, 2, 1)).astype(BF16_NP)
    G = ((Wq.T @ Wk) * SCALE).astype(BF16_NP)
    WvT = np.ascontiguousarray(Wv.T).astype(BF16_NP)
    nb = x.shape[0] // NCORES
    nc = _get_nc(nb)
    in_maps = [
        {"xT": xT[i * nb : (i + 1) * nb], "G": G, "WvT": WvT}
        for i in range(NCORES)
    ]
    res = run_bass_kernel_spmd(nc, in_maps, core_ids=list(range(NCORES)))
    return np.concatenate(
        [r["out"].astype(np.float32) for r in res.results], axis=0
    )


if __name__ == "__main__":
    rng = np.random.default_rng(0)
    x = rng.standard_normal((B, T, C), dtype=np.float32)
    s = 1.0 / np.sqrt(C)
    Wk = rng.standard_normal((H, C), dtype=np.float32) * s
    Wq = rng.standard_normal((H, C), dtype=np.float32) * s
    Wv = rng.standard_normal((H, C), dtype=np.float32) * s
    out = kernel(x=x, Wk=Wk, Wq=Wq, Wv=Wv)
    print(out.shape, out.dtype)
